# revision 35
# baseline (speedup 1.0000x reference)
"""Gaussian-mixture log-likelihood kernel for 8 Trainium2 NeuronCores.

Math: ll_i = ln Σ_j exp(d_ij + bias_j + C) - C, with
d_ij = -0.5 x_i^T A_j x_i + x_i^T m_j, A_j = S_j S_j^T, m_j = A_j c_j,
bias_j = ln(coef_j) - 0.5 c_j^T A_j c_j - threshold.

Layout is K-on-partitions: the PE contracts 576 feature rows per point
(512 circular-rotation pair products, 16 opposite-pair products, 32 linear
x rows, 3 bias ones-rows, 13 zero pad) against the cluster matrix B, giving
PSUM tiles [128 K-half, 512 points].  Everything on the contraction is fp8e4
with a x16 scale folded out in the Exp activation (scale=1/16), so the PE
runs DoubleRow perf mode (2 contraction rows per partition, 0.5 cyc/row).
The scalar engine exponentiates PSUM into an fp8 SBUF buffer; a second
DoubleRow matmul against a one-hot column (ones over the contraction dim)
reduces over all 256 clusters, accumulating each 512-point block's sums
into one persistent PSUM bank ([32, 512]).  A final Ln + scalar-add +
DMA-out produce 16384 log-likelihoods per core.

The pair-product features (x_i * x_b) are precomputed on host in float32
and shipped as fp8 (O(N D^2) work, ~0.4% of the N K D^2 device FLOPs),
which keeps the vector/gpsimd engines idle and the scalar engine (exp,
4.2M elems/core) as the single bottleneck.

Sharding: data-parallel over points, 16384 points/core; K-sized parameters
are replicated (precomputed on host in float64 - tiny vs the N*K work).
"""

import sys

sys.path.insert(0, "/opt/trn_rl_repo")

import numpy as np
import ml_dtypes

import concourse.bass as bass
import bass_rust
import concourse.bacc as bacc
import concourse.mybir as mybir
from concourse import bass_utils
from concourse.bass_interp import get_hw_module
from concourse.tile import TileContext

N, K, D = 131072, 256, 32
NCORES = 8
NC_PTS = N // NCORES            # 16384 points per core
F = 512                         # points per block (one PSUM bank of f32)
NBLK = NC_PTS // F              # 32 blocks
NROW = 576                      # feature rows = 3 pairs x 2 chunks x 96
BPG = 4                         # blocks per x2t DMA
SCALE = 16.0                    # fp8 B-side scale, undone by ACT scale=1/16
F32 = mybir.dt.float32
F8 = mybir.dt.float8e4
FP8_NP = ml_dtypes.float8_e4m3
DR = mybir.MatmulPerfMode.DoubleRow

_CACHE = {}


def _build(nc):
    x2t = nc.dram_tensor("x2t", [96, 6 * NC_PTS], F8, kind="ExternalInput").ap()
    bmat = nc.dram_tensor("bmat", [96, 6 * K], F8, kind="ExternalInput").ap()
    sel = nc.dram_tensor("sel", [128, 320], F8, kind="ExternalInput").ap()
    out = nc.dram_tensor("out", [32, F], F32, kind="ExternalOutput").ap()

    with TileContext(nc) as tc:
        with (
            tc.tile_pool(name="cst", bufs=1) as cpool,
            tc.tile_pool(name="xt", bufs=6) as xpool,
            tc.tile_pool(name="ebuf", bufs=1) as epool,
            tc.tile_pool(name="ps", bufs=1, space="PSUM") as ppool,
        ):
            # --- constants on SP/HWDGE (small, must land before the first
            # matmul chain); all x2t batches stream on Pool/SWDGE ---
            Bt = cpool.tile([96, 6, K], F8, tag="Bt")
            nc.sync.dma_start(
                out=Bt[:, :, :],
                in_=bass_rust.AP(bmat.tensor, 0, [(6 * K, 96), (K, 6), (1, K)]))
            selt = cpool.tile([128, 2, 160], F8, tag="sel")
            nc.sync.dma_start(
                out=selt[:, :, :],
                in_=bass_rust.AP(sel.tensor, 0, [(320, 128), (160, 2), (1, 160)]))

            e_all = epool.tile([128, 2 * NBLK, F], F8, tag="e_all")
            # 3 psum tiles x 2 banks rotate under 2-half ACT groups: a tile's
            # refill has a 2-period window, so the exp stream never stalls
            tiles = [ppool.tile([128, 2, F], F32, tag=f"t{i}", name=f"t{i}")
                     for i in range(3)]
            sums = ppool.tile([128, F], F32, tag="sums")

            # split the sums accumulation into two chains so blocks 0..29 can
            # be copied out and DMA'd while the last exps still run; only
            # blocks 30,31 remain on the critical-path tail
            SPLIT = NBLK - 2
            llE = cpool.tile([32, F], F32, tag="llE")
            llL = cpool.tile([32, F], F32, tag="llL")

            def emit_ones(r):
                nc.tensor.matmul(
                    out=sums[:, :],
                    lhsT=selt[:, :, 32 - r:160 - r],
                    rhs=e_all[:, 2 * r:2 * r + 2, :],
                    start=(r == 0 or r == SPLIT),
                    stop=(r == SPLIT - 1 or r == NBLK - 1),
                    perf_mode=DR)
                if r == SPLIT - 1:
                    nc.vector.tensor_copy(out=llE[:, :], in_=sums[0:32, :])
                    nc.sync.dma_start(out=out[0:SPLIT, :], in_=llE[0:SPLIT, :])

            ones_ptr = 0          # next block needing a ones-reduction
            exp_hi = -1           # highest half-index whose exp has been emitted

            def drain_ones(limit_half):
                # emit ones-reductions for blocks fully covered by exps
                # emitted at least one ACT instruction ago (lag keeps the PE
                # from head-of-line blocking on a still-running activation)
                nonlocal ones_ptr
                while ones_ptr < NBLK and 2 * ones_ptr + 1 <= limit_half:
                    emit_ones(ones_ptr)
                    ones_ptr += 1

            # x2t DMA batches (block-major HBM layout: [96, blk, 6, F]):
            # graduated sizes so the PE starts early and supply stays ahead
            dma_plan = [1, 2, 3] + [BPG] * 6 + [2]
            assert sum(dma_plan) == NBLK
            blk_map = {}
            b0 = 0
            for di, n in enumerate(dma_plan):
                for j in range(n):
                    blk_map[b0 + j] = (di, j, n, b0)
                b0 += n

            xt_tiles = {}
            prev_exp_hi = -1
            for b in range(NBLK):
                di, off, dsz, dblk = blk_map[b]
                if off == 0:
                    xt = xpool.tile([96, 6 * BPG, F], F8, tag="xt")
                    nc.gpsimd.dma_start(
                        out=xt[:, 0:6 * dsz, :],
                        in_=bass_rust.AP(x2t.tensor, dblk * 6 * F,
                                         [(6 * NC_PTS, 96), (1, dsz * 6 * F)]))
                    xt_tiles[di] = xt
                xt = xt_tiles[di]
                tile = tiles[b % 3]
                for h in range(2):
                    dst = tile[:, h:h + 1, :]
                    for q in range(3):
                        nc.tensor.matmul(
                            out=dst,
                            lhsT=Bt[:, 2 * q:2 * q + 2, 128 * h:128 * (h + 1)],
                            rhs=xt[:, 6 * off + 2 * q:6 * off + 2 * q + 2, :],
                            start=(q == 0), stop=(q == 2),
                            perf_mode=DR)
                nc.scalar.activation(
                    out=e_all[:, 2 * b:2 * b + 2, :],
                    in_=tile[:, :, :],
                    func=mybir.ActivationFunctionType.Exp,
                    scale=1.0 / SCALE)
                prev_exp_hi, exp_hi = exp_hi, 2 * b + 1
                drain_ones(prev_exp_hi)

            drain_ones(exp_hi)

            # ship raw per-point sums; ln(s) - C is O(N) host postprocessing
            # (avoids a 1.3us Ln act-table load on the critical-path tail)
            # parallel tail copy: DVE takes the low columns, ACT the high ones
            nc.vector.tensor_copy(out=llL[:, 0:F // 2], in_=sums[0:32, 0:F // 2])
            nc.scalar.copy(out=llL[:, F // 2:F], in_=sums[0:32, F // 2:F])
            nc.sync.dma_start(out=out[SPLIT:NBLK, :], in_=llL[SPLIT:NBLK, :])
    return nc


def _get_module():
    if "nc" not in _CACHE:
        nc = bacc.Bacc("TRN2", target_bir_lowering=False, debug=False,
                       num_devices=NCORES)
        _build(nc)
        nc.compile()
        nc.m = get_hw_module(nc.m)
        _CACHE["nc"] = nc
    return _CACHE["nc"]


def _fp8(x):
    return np.clip(x, -240.0, 240.0).astype(FP8_NP)


def _pack_rows(rows_by_part):
    # [576, n] -> [96, 6n] with row r = pair*192 + chunk*96 + p mapped to
    # partition p, free offset (2*pair + chunk)*n
    arr = rows_by_part.reshape(3, 2, 96, -1).transpose(2, 0, 1, 3)
    return np.ascontiguousarray(arr.reshape(96, -1))


def _host_params(centers, covs_inv_sqrt, weights, threshold):
    S = covs_inv_sqrt.astype(np.float64)
    w = np.abs(weights.astype(np.float64))
    cp = w / (w.sum() + 1e-30)
    A = np.einsum("kde,kfe->kdf", S, S)
    _, logdetA = np.linalg.slogdet(A)
    logcoef = np.log(np.maximum(cp, 1e-300)) + 0.5 * logdetA
    cen = centers.astype(np.float64)
    m = np.einsum("kde,ke->kd", A, cen)
    t_cAc = np.einsum("kd,kd->k", m, cen)
    thr = float(threshold[0])
    bias0 = logcoef - 0.5 * t_cAc - thr
    C = 4.0 - bias0.max()
    b16 = SCALE * (bias0 + C)

    Brows = np.zeros((NROW, K))
    for o in range(16):
        f = -0.5 if o == 0 else -1.0
        for i in range(32):
            Brows[32 * o + i] = f * SCALE * A[:, i, (i + o) % 32]
    for i in range(16):
        Brows[512 + i] = -SCALE * A[:, i, i + 16]
    for i in range(32):
        Brows[528 + i] = SCALE * m[:, i]
    hi = _fp8(b16).astype(np.float64)
    mid = _fp8(b16 - hi).astype(np.float64)
    lo = _fp8(b16 - hi - mid).astype(np.float64)
    Brows[560], Brows[561], Brows[562] = hi, mid, lo
    return _pack_rows(_fp8(Brows.astype(np.float32))), np.float32(-C)


def _host_x2t(pts):
    # pts [NC_PTS, 32] f32 -> [96, 6*NC_PTS] fp8 feature rows, block-major:
    # partition p, free offset ((blk*3 + pair)*2 + chunk)*F + f
    xT = np.ascontiguousarray(pts.T)               # [32, n]
    n = xT.shape[1]
    rows = np.empty((NROW, n), np.float32)
    for o in range(16):
        rows[32 * o:32 * o + 32] = xT * np.roll(xT, -o, axis=0)
    rows[512:528] = xT[:16] * xT[16:]
    rows[528:560] = xT
    rows[560:563] = 1.0
    rows[563:576] = 0.0
    arr = _fp8(rows).reshape(3, 2, 96, n // F, F).transpose(2, 3, 0, 1, 4)
    return np.ascontiguousarray(arr.reshape(96, -1))


def kernel(points, centers, covs_inv_sqrt, weights, threshold):
    points = np.asarray(points, dtype=np.float32)
    Bpk, negC = _host_params(np.asarray(centers), np.asarray(covs_inv_sqrt),
                             np.asarray(weights), np.asarray(threshold))
    selh = np.zeros((128, 320), np.float32)
    selh[:, 32] = 1.0
    selh[:, 192] = 1.0
    selh = selh.astype(FP8_NP)

    in_maps = []
    for r in range(NCORES):
        x2t = _host_x2t(points[r * NC_PTS:(r + 1) * NC_PTS])
        in_maps.append({"x2t": x2t, "bmat": Bpk, "sel": selh})

    nc = _get_module()
    res = bass_utils.run_bass_kernel_spmd(nc, in_maps,
                                          core_ids=list(range(NCORES)))
    s = np.concatenate([res.results[r]["out"].reshape(-1)
                        for r in range(NCORES)])
    ll = np.log(s.astype(np.float64)) + np.float64(negC)
    return ll.reshape(N, 1).astype(np.float32)
